# revision 30
# baseline (speedup 1.0000x reference)
"""BoneLinear Trainium2 kernel (8-core SPMD, data-parallel over batch).

Math: reference computes out = x @ (weight + w)^T where w is the bone
block-update of weight:
    wblk = weight.reshape(a, r, b, r).transpose(0,2,1,3)      # (a,b,r,r)
    wup  = wblk @ bone[b] + bone[b]                            # per (a,b)
    w    = wup.transpose(0,2,1,3).reshape(out_f, in_f)

Identity used here (verified numerically): with y[:, b*r:(b+1)*r] =
x[:, b*r:(b+1)*r] @ bone[b].T and s = sum_b y[:, b-block]:

    out = (x + y) @ weight^T + tile(s over out-blocks)

so the heavy GEMM uses the *original* weight; the bone update reduces to a
cheap block-diagonal transform of x plus a rank-64 broadcast correction.

Per core (batch element): z^T = x^T + blockdiag(bone^T) @ x^T is computed on
the PE in t-quarters and kept SBUF-resident in fp16; the main GEMM
out = z^T.T @ W^T streams W^T from HBM; s is accumulated on the PE and added
(broadcast over 64-column blocks) during PSUM eviction.
"""

import numpy as np

B, T, IN, OUT, R = 8, 2048, 4096, 4096, 64
P = 128
KT = IN // P  # 32 contraction tiles
TQ = 512  # t-quarter size
NQ = T // TQ  # 4 quarters
NFREE = 512  # matmul moving free dim / o-tile size
OTN = OUT // NFREE  # 8 o-tiles

_NC_CACHE = {}


def _build_nc(
    reps=1,
    nfree=NFREE,
    po_bufs=4,
    py_bufs=2,
    xt_bufs=36,
    wt_on_act=True,
    act_copy=True,
    xt_chunk=1,
    interleave_p1=True,
):
    import concourse.mybir as mybir
    from concourse import bacc
    from concourse.tile import TileContext
    from concourse.masks import make_identity

    F16 = mybir.dt.float16
    F32 = mybir.dt.float32
    otn = OUT // nfree

    nc = bacc.Bacc(None, target_bir_lowering=False)
    xT = nc.dram_tensor("xT", [IN, T], F16, kind="ExternalInput")
    wT = nc.dram_tensor("wT", [IN, OUT], F16, kind="ExternalInput")
    bd = nc.dram_tensor("bd", [P, KT, P], F16, kind="ExternalInput")
    bdv = nc.dram_tensor("bdv", [P, KT, R], F16, kind="ExternalInput")
    out = nc.dram_tensor("out", [T, OUT], F32, kind="ExternalOutput")

    wTv = wT.rearrange("(ko p) o -> p ko o", p=P)

    with TileContext(nc) as tc:
        with (
            tc.tile_pool(name="const", bufs=1) as constp,
            tc.tile_pool(name="xt", bufs=xt_bufs) as xpool,
            tc.tile_pool(name="zt", bufs=2) as zpool,
            tc.tile_pool(name="wt", bufs=2) as wpool,
            tc.tile_pool(name="sb", bufs=2) as spool,
            tc.tile_pool(name="ob", bufs=3) as opool,
            tc.tile_pool(name="py", bufs=py_bufs, space="PSUM") as pyp,
            tc.tile_pool(name="ps", bufs=1, space="PSUM") as psp,
            tc.tile_pool(name="po", bufs=po_bufs, space="PSUM") as pop,
        ):
            bd_sb = constp.tile([P, KT, P], F16, tag="bd")
            nc.sync.dma_start(bd_sb[:], bd[:])
            bdv_sb = constp.tile([P, KT, R], F16, tag="bdv")
            nc.sync.dma_start(bdv_sb[:], bdv[:])
            ident = constp.tile([R, R], F32, tag="ident")
            make_identity(nc, ident)

            # Next-quarter xt tiles are prefetched during the current
            # quarter's phase 2, so phase 1 never waits on DMA (and the small
            # xt loads don't get stuck behind a 4MB wt transfer at the
            # quarter boundary). xt_chunk>1 batches that many k-tiles per
            # DMA (fewer, larger transfers).
            xt_tiles = {}
            xTv = xT.rearrange("(ko p) t -> p ko t", p=P)

            def prefetch_xt(qi):
                tq0 = (qi % NQ) * TQ
                tiles = []
                for kc in range(0, KT, xt_chunk):
                    xt = xpool.tile([P, xt_chunk, TQ], F16, tag="xt")
                    nc.sync.dma_start(
                        xt[:], xTv[:, kc : kc + xt_chunk, tq0 : tq0 + TQ]
                    )
                    for j in range(xt_chunk):
                        tiles.append(xt[:, j, :])
                xt_tiles[qi] = tiles

            def build_phase1(qi, nchunks):
                """Allocate quarter-qi phase-1 tiles. Returns (zt, s_sb, emit)
                where emit(ci) emits chunk ci of the k-loop; the last chunk
                also emits the s finalize (fold + transpose to t-layout)."""
                zt = zpool.tile([P, KT, TQ], F16, tag="zt")
                s_sb = spool.tile([P, TQ // P, R], F32, tag="ssb")
                psum_s = psp.tile([R, TQ], F32, tag="ps")
                xts = xt_tiles.pop(qi)
                per = KT // nchunks

                def emit(ci):
                    for k in range(ci * per, (ci + 1) * per):
                        xt = xts[k]
                        py = pyp.tile([P, TQ], F32, tag="py")
                        # z^T tile directly: blockdiag(I + bone[2k]^T, ...)
                        # @ x^T (x rides the identity exactly — same rounding
                        # as an explicit fp32 add of fp16 x).
                        nc.tensor.matmul(
                            py[:], bd_sb[:, k, :], xt[:], start=True, stop=True
                        )
                        # s^T accumulation: vstack(bone[2k]^T, bone[2k+1]^T)
                        nc.tensor.matmul(
                            psum_s[:],
                            bdv_sb[:, k, :],
                            xt[:],
                            start=(k == 0),
                            stop=(k == KT - 1),
                        )
                        # psum -> SBUF fp16; split 2:1 across DVE and the
                        # idle ACT engine so copies keep pace with the PE.
                        if act_copy and k % 3 == 2:
                            nc.scalar.copy(zt[:, k, :], py[:])
                        else:
                            nc.vector.tensor_copy(zt[:, k, :], py[:])
                    if ci == nchunks - 1:
                        # s: [R, TQ] -> t-partition layout [P, TQ//P, R]
                        sT = spool.tile([R, TQ], F32, tag="sT")
                        nc.vector.tensor_copy(sT[:], psum_s[:])
                        for c in range(TQ // P):
                            pt = pyp.tile([P, R], F32, tag="py")
                            nc.tensor.transpose(
                                pt[:], sT[:, c * P : (c + 1) * P], ident[:]
                            )
                            nc.vector.tensor_copy(s_sb[:, c, :], pt[:])

                return zt, s_sb, emit

            # wt DMAs ride the ACT HWDGE ring (wt_on_act) so the next
            # quarter's first weight tile isn't FIFO-queued behind the 32
            # xt loads on the SP ring — hides the 4MB load under compute.
            wt_dma = nc.scalar.dma_start if wt_on_act else nc.sync.dma_start

            # reps>1 repeats the whole computation (timing builds only —
            # wall-time differencing cancels host/transfer overhead).
            prefetch_xt(0)
            cur = build_phase1(0, 1)
            cur[2](0)
            for qi in range(NQ * reps):
                q = qi % NQ
                t0 = q * TQ
                zt, s_sb, _ = cur
                nxt = None
                # ---- phase 2: out quarter = z^T.T @ W^T + s ----
                # Phase 1 of quarter qi+1 is emitted in chunks between this
                # quarter's o-tiles (interleave_p1), so its short matmuls and
                # psum->SBUF copies hide inside the dense GEMM stream instead
                # of forming a serial DVE/ACT-paced wall at the boundary.
                for ot in range(otn):
                    wt = wpool.tile([P, KT, nfree], F16, tag="wt")
                    wt_dma(wt[:], wTv[:, :, ot * nfree : (ot + 1) * nfree])
                    if ot == 0 and qi + 1 < NQ * reps:
                        # Next quarter's x loads: emitted after this quarter's
                        # first weight tile so the SP ring serves wt first.
                        prefetch_xt(qi + 1)
                        if interleave_p1 == 2:
                            nxt = build_phase1(qi + 1, otn * (TQ // P))
                        elif interleave_p1:
                            nxt = build_phase1(qi + 1, otn)
                    for tt in range(TQ // P):
                        po = pop.tile([P, nfree], F32, tag="po")
                        for k in range(KT):
                            nc.tensor.matmul(
                                po[:],
                                zt[:, k, tt * P : (tt + 1) * P],
                                wt[:, k, :],
                                start=(k == 0),
                                stop=(k == KT - 1),
                            )
                        ob = opool.tile([P, nfree], F32, tag="ob")
                        ob3 = ob.rearrange("p (a r) -> p a r", r=R)
                        po3 = po.rearrange("p (a r) -> p a r", r=R)
                        s_bcast = s_sb[:, tt, :][:, None, :].to_broadcast(
                            (P, nfree // R, R)
                        )
                        nc.vector.tensor_add(ob3, po3, s_bcast)
                        nc.sync.dma_start(
                            out[
                                t0 + tt * P : t0 + (tt + 1) * P,
                                ot * nfree : (ot + 1) * nfree,
                            ],
                            ob[:],
                        )
                        if nxt is not None and interleave_p1 == 2:
                            nxt[2](ot * (TQ // P) + tt)
                    if nxt is not None and interleave_p1 != 2:
                        nxt[2](ot)
                if qi + 1 < NQ * reps and not interleave_p1:
                    nxt = build_phase1(qi + 1, 1)
                    nxt[2](0)
                cur = nxt
    nc.compile()
    return nc


def _get_nc(reps=1):
    key = ("nc", reps)
    if key not in _NC_CACHE:
        _NC_CACHE[key] = _build_nc(reps)
    return _NC_CACHE[key]


def prep_in_maps(x, weight, bone):
    """Host-side layout prep: transposes + block placement + fp16 cast."""
    x = np.asarray(x, dtype=np.float32)
    weight = np.asarray(weight, dtype=np.float32)
    bone = np.asarray(bone, dtype=np.float32)
    assert x.shape == (B, T, IN), x.shape
    assert weight.shape == (OUT, IN), weight.shape
    assert bone.shape == (IN // R, R, R), bone.shape

    wT16 = np.ascontiguousarray(weight.T).astype(np.float16)
    boneT = bone.transpose(0, 2, 1).astype(np.float16)  # bone[b]^T
    bdmat = np.zeros((KT, P, P), np.float16)
    bdmat[:, 0:R, 0:R] = boneT[0::2]
    bdmat[:, R:P, R:P] = boneT[1::2]
    bdmat += np.eye(P, dtype=np.float16)[None]  # fold the +x into the y-mm
    bd_host = np.ascontiguousarray(bdmat.transpose(1, 0, 2))  # [P, KT, P]
    bdvm = np.zeros((KT, P, R), np.float16)
    bdvm[:, 0:R, :] = boneT[0::2]
    bdvm[:, R:P, :] = boneT[1::2]
    bdv_host = np.ascontiguousarray(bdvm.transpose(1, 0, 2))  # [P, KT, R]

    in_maps = []
    for i in range(B):
        xT16 = np.ascontiguousarray(x[i].T).astype(np.float16)
        in_maps.append({"xT": xT16, "wT": wT16, "bd": bd_host, "bdv": bdv_host})
    return in_maps


def kernel(x, weight, bone):
    from concourse.bass_utils import run_bass_kernel_spmd

    nc = _get_nc()
    in_maps = prep_in_maps(x, weight, bone)
    res = run_bass_kernel_spmd(nc, in_maps, core_ids=list(range(B)))
    return np.stack([r["out"] for r in res.results], axis=0)


if __name__ == "__main__":
    rng = np.random.default_rng(0)
    x = rng.standard_normal((B, T, IN), dtype=np.float32)
    weight = (rng.standard_normal((OUT, IN)) * 0.02).astype(np.float32)
    bone = (rng.standard_normal((IN // R, R, R)) * 0.02).astype(np.float32)
    out = kernel(x=x, weight=weight, bone=bone)
    print(out.shape, out.dtype)


# revision 35
# speedup vs baseline: 1.7993x; 1.7993x over previous
"""BoneLinear Trainium2 kernel (8-core SPMD, data-parallel over batch).

Math: reference computes out = x @ (weight + w)^T where w is the bone
block-update of weight:
    wblk = weight.reshape(a, r, b, r).transpose(0,2,1,3)      # (a,b,r,r)
    wup  = wblk @ bone[b] + bone[b]                            # per (a,b)
    w    = wup.transpose(0,2,1,3).reshape(out_f, in_f)

Identity used here (verified numerically): with y[:, b*r:(b+1)*r] =
x[:, b*r:(b+1)*r] @ bone[b].T and s = sum_b y[:, b-block]:

    out = (x + y) @ weight^T + tile(s over out-blocks)

so the heavy GEMM uses the *original* weight; the bone update reduces to a
cheap block-diagonal transform of x plus a rank-64 broadcast correction.

Per core (batch element): z^T = x^T + blockdiag(bone^T) @ x^T is computed on
the PE in t-quarters and kept SBUF-resident in fp16; the main GEMM
out = z^T.T @ W^T streams W^T from HBM; s is accumulated on the PE and added
(broadcast over 64-column blocks) during PSUM eviction.
"""

import numpy as np

B, T, IN, OUT, R = 8, 2048, 4096, 4096, 64
P = 128
KT = IN // P  # 32 contraction tiles
TQ = 512  # t-quarter size
NQ = T // TQ  # 4 quarters
NFREE = 512  # matmul moving free dim / o-tile size
OTN = OUT // NFREE  # 8 o-tiles

_NC_CACHE = {}


def _build_nc(
    reps=1,
    nfree=NFREE,
    po_bufs=4,
    py_bufs=2,
    xt_bufs=36,
    wt_on_act=True,
    act_copy=True,
    xt_chunk=1,
    interleave_p1=True,
    fuse_start=True,
):
    import concourse.mybir as mybir
    from concourse import bacc
    from concourse.tile import TileContext
    from concourse.masks import make_identity

    F16 = mybir.dt.float16
    F32 = mybir.dt.float32
    otn = OUT // nfree

    nc = bacc.Bacc(None, target_bir_lowering=False)
    xT = nc.dram_tensor("xT", [IN, T], F16, kind="ExternalInput")
    wT = nc.dram_tensor("wT", [IN, OUT], F16, kind="ExternalInput")
    bd = nc.dram_tensor("bd", [P, KT, P], F16, kind="ExternalInput")
    bdv = nc.dram_tensor("bdv", [P, KT, R], F16, kind="ExternalInput")
    out = nc.dram_tensor("out", [T, OUT], F32, kind="ExternalOutput")

    wTv = wT.rearrange("(ko p) o -> p ko o", p=P)

    with TileContext(nc) as tc:
        with (
            tc.tile_pool(name="const", bufs=1) as constp,
            tc.tile_pool(name="xt", bufs=xt_bufs) as xpool,
            tc.tile_pool(name="zt", bufs=2) as zpool,
            tc.tile_pool(name="wt", bufs=2) as wpool,
            tc.tile_pool(name="sb", bufs=2) as spool,
            tc.tile_pool(name="ob", bufs=3) as opool,
            tc.tile_pool(name="py", bufs=py_bufs, space="PSUM") as pyp,
            tc.tile_pool(name="ps", bufs=1, space="PSUM") as psp,
            tc.tile_pool(name="po", bufs=po_bufs, space="PSUM") as pop,
        ):
            bd_sb = constp.tile([P, KT, P], F16, tag="bd")
            nc.sync.dma_start(bd_sb[:], bd[:])
            bdv_sb = constp.tile([P, KT, R], F16, tag="bdv")
            nc.sync.dma_start(bdv_sb[:], bdv[:])
            ident = constp.tile([R, R], F32, tag="ident")
            make_identity(nc, ident)

            # Next-quarter xt tiles are prefetched during the current
            # quarter's phase 2, so phase 1 never waits on DMA (and the small
            # xt loads don't get stuck behind a 4MB wt transfer at the
            # quarter boundary). xt_chunk>1 batches that many k-tiles per
            # DMA (fewer, larger transfers).
            xt_tiles = {}
            xTv = xT.rearrange("(ko p) t -> p ko t", p=P)

            def prefetch_xt(qi):
                tq0 = (qi % NQ) * TQ
                tiles = []
                for kc in range(0, KT, xt_chunk):
                    xt = xpool.tile([P, xt_chunk, TQ], F16, tag="xt")
                    nc.sync.dma_start(
                        xt[:], xTv[:, kc : kc + xt_chunk, tq0 : tq0 + TQ]
                    )
                    for j in range(xt_chunk):
                        tiles.append(xt[:, j, :])
                xt_tiles[qi] = tiles

            def build_phase1(qi, nchunks):
                """Allocate quarter-qi phase-1 tiles. Returns (zt, s_sb, emit)
                where emit(ci) emits chunk ci of the k-loop; the last chunk
                also emits the s finalize (fold + transpose to t-layout)."""
                zt = zpool.tile([P, KT, TQ], F16, tag="zt")
                s_sb = spool.tile([P, TQ // P, R], F32, tag="ssb")
                psum_s = psp.tile([R, TQ], F32, tag="ps")
                xts = xt_tiles.pop(qi)
                per = KT // nchunks

                def emit(ci):
                    for k in range(ci * per, (ci + 1) * per):
                        xt = xts[k]
                        py = pyp.tile([P, TQ], F32, tag="py")
                        # z^T tile directly: blockdiag(I + bone[2k]^T, ...)
                        # @ x^T (x rides the identity exactly — same rounding
                        # as an explicit fp32 add of fp16 x).
                        nc.tensor.matmul(
                            py[:], bd_sb[:, k, :], xt[:], start=True, stop=True
                        )
                        # s^T accumulation: vstack(bone[2k]^T, bone[2k+1]^T)
                        nc.tensor.matmul(
                            psum_s[:],
                            bdv_sb[:, k, :],
                            xt[:],
                            start=(k == 0),
                            stop=(k == KT - 1),
                        )
                        # psum -> SBUF fp16; split 2:1 across DVE and the
                        # idle ACT engine so copies keep pace with the PE.
                        if act_copy and k % 3 == 2:
                            nc.scalar.copy(zt[:, k, :], py[:])
                        else:
                            nc.vector.tensor_copy(zt[:, k, :], py[:])
                    if ci == nchunks - 1:
                        # s: [R, TQ] -> t-partition layout [P, TQ//P, R]
                        sT = spool.tile([R, TQ], F32, tag="sT")
                        nc.vector.tensor_copy(sT[:], psum_s[:])
                        for c in range(TQ // P):
                            pt = pyp.tile([P, R], F32, tag="py")
                            nc.tensor.transpose(
                                pt[:], sT[:, c * P : (c + 1) * P], ident[:]
                            )
                            nc.vector.tensor_copy(s_sb[:, c, :], pt[:])

                return zt, s_sb, emit

            # wt DMAs ride the ACT HWDGE ring (wt_on_act) so the next
            # quarter's first weight tile isn't FIFO-queued behind the 32
            # xt loads on the SP ring — hides the 4MB load under compute.
            wt_dma = nc.scalar.dma_start if wt_on_act else nc.sync.dma_start

            # reps>1 repeats the whole computation (timing builds only —
            # wall-time differencing cancels host/transfer overhead).
            prefetch_xt(0)
            if fuse_start:
                # Quarter 0's phase 1 is emitted k-by-k inside its first
                # o-tile (see below) so the PE stream stays dense from the
                # start instead of idling through a DMA-paced prologue.
                cur = build_phase1(0, KT)
            else:
                cur = build_phase1(0, 1)
                cur[2](0)
            for qi in range(NQ * reps):
                q = qi % NQ
                t0 = q * TQ
                zt, s_sb, _ = cur
                nxt = None
                # ---- phase 2: out quarter = z^T.T @ W^T + s ----
                # Phase 1 of quarter qi+1 is emitted in chunks between this
                # quarter's o-tiles (interleave_p1), so its short matmuls and
                # psum->SBUF copies hide inside the dense GEMM stream instead
                # of forming a serial DVE/ACT-paced wall at the boundary.
                for ot in range(otn):
                    wt = wpool.tile([P, KT, nfree], F16, tag="wt")
                    if qi == 0 and ot == 0 and fuse_start:
                        # Chunked so the fused k-loop's first matmuls don't
                        # gate on the full 4MB transfer.
                        for kc in range(0, KT, 8):
                            wt_dma(
                                wt[:, kc : kc + 8, :],
                                wTv[:, kc : kc + 8, ot * nfree : (ot + 1) * nfree],
                            )
                    else:
                        wt_dma(wt[:], wTv[:, :, ot * nfree : (ot + 1) * nfree])
                    if ot == 0 and qi + 1 < NQ * reps:
                        # Next quarter's x loads: emitted after this quarter's
                        # first weight tile so the SP ring serves wt first.
                        prefetch_xt(qi + 1)
                        if interleave_p1 == 2:
                            nxt = build_phase1(qi + 1, otn * (TQ // P))
                        elif interleave_p1:
                            nxt = build_phase1(qi + 1, otn)
                    if qi == 0 and ot == 0 and fuse_start:
                        # k-outer / tt-inner: emit phase-1 step k, then the
                        # four psum-group matmuls that consume zt[:, k].
                        pos = []
                        for _tt in range(TQ // P):
                            po_f = pop.tile([P, nfree], F32, tag="po")
                            pos.append(po_f)
                        for k in range(KT):
                            cur[2](k)
                            for tt in range(TQ // P):
                                nc.tensor.matmul(
                                    pos[tt][:],
                                    zt[:, k, tt * P : (tt + 1) * P],
                                    wt[:, k, :],
                                    start=(k == 0),
                                    stop=(k == KT - 1),
                                )
                        for tt in range(TQ // P):
                            ob = opool.tile([P, nfree], F32, tag="ob")
                            ob3 = ob.rearrange("p (a r) -> p a r", r=R)
                            po3 = pos[tt].rearrange("p (a r) -> p a r", r=R)
                            s_bcast = s_sb[:, tt, :][:, None, :].to_broadcast(
                                (P, nfree // R, R)
                            )
                            nc.vector.tensor_add(ob3, po3, s_bcast)
                            nc.sync.dma_start(
                                out[
                                    t0 + tt * P : t0 + (tt + 1) * P,
                                    ot * nfree : (ot + 1) * nfree,
                                ],
                                ob[:],
                            )
                        if nxt is not None and interleave_p1 != 2:
                            nxt[2](ot)
                        continue
                    for tt in range(TQ // P):
                        po = pop.tile([P, nfree], F32, tag="po")
                        for k in range(KT):
                            nc.tensor.matmul(
                                po[:],
                                zt[:, k, tt * P : (tt + 1) * P],
                                wt[:, k, :],
                                start=(k == 0),
                                stop=(k == KT - 1),
                            )
                        ob = opool.tile([P, nfree], F32, tag="ob")
                        ob3 = ob.rearrange("p (a r) -> p a r", r=R)
                        po3 = po.rearrange("p (a r) -> p a r", r=R)
                        s_bcast = s_sb[:, tt, :][:, None, :].to_broadcast(
                            (P, nfree // R, R)
                        )
                        nc.vector.tensor_add(ob3, po3, s_bcast)
                        nc.sync.dma_start(
                            out[
                                t0 + tt * P : t0 + (tt + 1) * P,
                                ot * nfree : (ot + 1) * nfree,
                            ],
                            ob[:],
                        )
                        if nxt is not None and interleave_p1 == 2:
                            nxt[2](ot * (TQ // P) + tt)
                    if nxt is not None and interleave_p1 != 2:
                        nxt[2](ot)
                if qi + 1 < NQ * reps and not interleave_p1:
                    nxt = build_phase1(qi + 1, 1)
                    nxt[2](0)
                cur = nxt
    nc.compile()
    return nc


def _get_nc(reps=1):
    key = ("nc", reps)
    if key not in _NC_CACHE:
        _NC_CACHE[key] = _build_nc(reps)
    return _NC_CACHE[key]


def prep_in_maps(x, weight, bone):
    """Host-side layout prep: transposes + block placement + fp16 cast."""
    x = np.asarray(x, dtype=np.float32)
    weight = np.asarray(weight, dtype=np.float32)
    bone = np.asarray(bone, dtype=np.float32)
    assert x.shape == (B, T, IN), x.shape
    assert weight.shape == (OUT, IN), weight.shape
    assert bone.shape == (IN // R, R, R), bone.shape

    wT16 = np.ascontiguousarray(weight.T).astype(np.float16)
    boneT = bone.transpose(0, 2, 1).astype(np.float16)  # bone[b]^T
    bdmat = np.zeros((KT, P, P), np.float16)
    bdmat[:, 0:R, 0:R] = boneT[0::2]
    bdmat[:, R:P, R:P] = boneT[1::2]
    bdmat += np.eye(P, dtype=np.float16)[None]  # fold the +x into the y-mm
    bd_host = np.ascontiguousarray(bdmat.transpose(1, 0, 2))  # [P, KT, P]
    bdvm = np.zeros((KT, P, R), np.float16)
    bdvm[:, 0:R, :] = boneT[0::2]
    bdvm[:, R:P, :] = boneT[1::2]
    bdv_host = np.ascontiguousarray(bdvm.transpose(1, 0, 2))  # [P, KT, R]

    in_maps = []
    for i in range(B):
        xT16 = np.ascontiguousarray(x[i].T).astype(np.float16)
        in_maps.append({"xT": xT16, "wT": wT16, "bd": bd_host, "bdv": bdv_host})
    return in_maps


def kernel(x, weight, bone):
    from concourse.bass_utils import run_bass_kernel_spmd

    nc = _get_nc()
    in_maps = prep_in_maps(x, weight, bone)
    res = run_bass_kernel_spmd(nc, in_maps, core_ids=list(range(B)))
    return np.stack([r["out"] for r in res.results], axis=0)


if __name__ == "__main__":
    rng = np.random.default_rng(0)
    x = rng.standard_normal((B, T, IN), dtype=np.float32)
    weight = (rng.standard_normal((OUT, IN)) * 0.02).astype(np.float32)
    bone = (rng.standard_normal((IN // R, R, R)) * 0.02).astype(np.float32)
    out = kernel(x=x, weight=weight, bone=bone)
    print(out.shape, out.dtype)
